# revision 34
# baseline (speedup 1.0000x reference)
"""ConditionalSelfAttention (B=8, C=256, H=W=64, QK=32, LC=32) on 8 TRN2 NeuronCores.

Data-parallel over batch: core b computes batch element b.

Two device programs, selected at runtime on the value of gamma:
  * gamma == 0 (the reference's init state): out = concat(x, broadcast(u))
    exactly -- the attention branch is multiplied by zero, so the kernel is a
    pure HBM-bandwidth problem (~4.6 MB traffic/core, ~19-21 us incl. ~9 us
    fixed NEFF pre/postamble). Fast program:
      - sync HWDGE ring: labelpack load (1 packed [128,128] f32 tensor; 512 B
        per partition -- separate small loads degrade to 4 B descriptors),
        then the 2 MB DRAM->DRAM bf16 copy of x into out rows 0:256. One data
        ring only: packet-granular round-robin across rings starves the ring
        holding small packets.
      - label branch u = softmax(label) @ We.T + be on DVE/ScalarE, computed
        4x-replicated over 128 partitions (no max-shift; ACT exp-table
        prefetched via a dummy activation), broadcast to [128, 1024] bf16,
        one 256 KB store on the scalar HWDGE ring.
  * gamma != 0: the full attention program below (as inherited; note its
    fp8e5 softmax quantization costs ~10% rel err at gamma=O(1)).

Per-core program (fp8 DoubleRow AV, [c,i] output layout, padded-K energy):
  xf (256, 4096) fp32 (+ bf16 copy for matmul inputs), DMA'd in 4 n-quarters
  q4 = Wq@xf + bq (bf16, 4x-replicated rows); k_pad = Wk@xf + bk in rows 0:32,
    rows 32:128 zeroed  (K=128 plain matmuls -- tile_position row packing is
    incompatible with DoubleRow matmuls in one NEFF, crashes the exec unit)
  energyT[j, i] = sum_d k_pad[d, j] q4[d, i]   (plain K=128, zero rows inert)
  PT = exp(energyT - SHIFT) -> fp8e5           (ScalarE, PSUM -> SBUF)
  vt[u] = fp8e4 [128, 2, 256]: vt[u][p, t, c] = v[c, (2u+t)*128+p]
  o_ps[c, i] = sum_{p,t} vt[p,t,c] PT[p,t,i]   (fp8 DoubleRow matmuls, K=256)
  l_bc[*, i] = sum_{p,t} 1 * PT[p,t,i]         (all-ones DR stationary)
  out[c, i] = gamma/l_i * o_ps + (gamma*bv_c + xf[c, i])  (2 fused DVE stt ops)
  rows 256:288 = broadcast(softmax(label) @ We.T + be)    (DVE+GpSimd, no PE)
"""

import numpy as np

import concourse.bass as bass
import concourse.bacc as bacc
import concourse.mybir as mybir
import concourse.tile as tile
from concourse.bass_utils import run_bass_kernel_spmd

F32 = mybir.dt.float32
BF16 = mybir.dt.bfloat16
FP8E5 = mybir.dt.float8e5
FP8E4 = mybir.dt.float8e4
AF = mybir.ActivationFunctionType
ALU = mybir.AluOpType
DR = mybir.MatmulPerfMode.DoubleRow

B, C, HW, N = 8, 256, 64, 4096
QK, LC = 32, 32
COUT = C + LC  # 288
SHIFT = 28.0
EPS = 1e-30

IC = 512          # i-chunk for the energy/exp/AV phase
NIC = N // IC     # 8
NJB = N // 128    # 32 j-blocks
NPAIR = NJB // 2  # 16 j-block pairs (DoubleRow contracts 256 j per matmul)
NQT = 4           # x DMA quarters


def host_prep(x_b, label_b, Wq, bq, Wk, bk, Wv, bv, gamma, We, be):
    """Per-core input dict. x_b: (C, H, W); label_b: (LC,)."""
    import ml_dtypes
    xb = np.ascontiguousarray(
        x_b.reshape(C, N).astype(np.float32).astype(ml_dtypes.bfloat16))
    wq4 = np.ascontiguousarray(np.tile(Wq.T, (1, 4)).astype(np.float32))
    wk4 = np.ascontiguousarray(np.tile(Wk.T, (1, 4)).astype(np.float32))
    bq4 = np.ascontiguousarray(np.tile(bq, 4)[:, None].astype(np.float32))
    bk4 = np.ascontiguousarray(np.tile(bk, 4)[:, None].astype(np.float32))
    wv_t = np.ascontiguousarray(Wv.T.astype(np.float32))
    g = np.float32(np.asarray(gamma).reshape(-1)[0])
    gbv = np.ascontiguousarray((g * bv.astype(np.float32)).reshape(C, 1))
    return {
        "x": xb,
        "wq4": wq4,
        "wk4": wk4,
        "bq4": bq4,
        "bk4": bk4,
        "wv_t": wv_t,
        "gbv": gbv,
        "we32": np.ascontiguousarray(We.astype(np.float32)),
        "be_col": np.ascontiguousarray(be[:, None].astype(np.float32)),
        "label": np.ascontiguousarray(
            np.tile(label_b[None, :].astype(np.float32), (LC, 1))),
        "gamma": np.ascontiguousarray(np.asarray(gamma, np.float32).reshape(1, 1)),
    }


def host_prep_fast(x_b, label_b, We, be):
    """Per-core input dict for the gamma==0 program."""
    import ml_dtypes
    xb = np.ascontiguousarray(
        x_b.reshape(C, N).astype(np.float32).astype(ml_dtypes.bfloat16))
    # partition p of the label branch computes u[p // 4] (4x replication so the
    # final [128, 1024] store covers out rows 256:288 in one DMA). All inputs
    # packed into one [128, 128] f32 tensor (512 B/partition: single DMA at
    # line rate; separate loads degrade to 4-byte descriptors).
    pk = np.zeros((128, 128), np.float32)
    pk[:, 0:LC] = label_b.astype(np.float32)[None, :]
    pk[:, LC:2 * LC] = np.repeat(We.astype(np.float32), 4, axis=0)
    pk[:, 2 * LC] = np.repeat(be.astype(np.float32), 4)
    return {"x": xb, "labelpack": np.ascontiguousarray(pk)}


def build_fast_program(nc, tc):
    """gamma == 0: out rows 0:256 = x (copy), rows 256:288 = label branch."""
    x_d = nc.dram_tensor("x", [C, N], BF16, kind="ExternalInput")
    pk_d = nc.dram_tensor("labelpack", [128, 128], F32, kind="ExternalInput")
    out_d = nc.dram_tensor("out", [COUT, N], BF16, kind="ExternalOutput")

    from contextlib import ExitStack
    ctx = ExitStack()
    pool = ctx.enter_context(tc.tile_pool(name="fast", bufs=1))

    # ACT exp-table prefetch: a dummy activation on an early-ready tile pulls
    # the ~1.3us ACT_TABLE_LOAD off the label-branch critical path
    dz = pool.tile([LC, 1], F32, name="dz")
    nc.vector.memset(dz[:], 0.0)
    dummy = pool.tile([LC, 1], F32, name="dummy")
    nc.scalar.activation(dummy[:], dz[:], AF.Exp, scale=1.0)

    # label-branch input: one packed load at the HEAD of the sync ring, so
    # its 128 small descriptors drain exclusively (~1us) before the bulk copy
    # occupies the SDMA engines. SWDGE/gpsimd is avoided entirely -- its
    # descriptor path is slower and adds a multi-us queue drain at the end.
    pk = pool.tile([128, 128], F32, name="pk")
    nc.sync.dma_start(pk[:], pk_d[:])
    lbl = pk[:, 0:LC]
    we = pk[:, LC:2 * LC]
    be = pk[:, 2 * LC:2 * LC + 1]

    # bulk x copy: DRAM->DRAM on the sync HWDGE ring, behind the labelpack
    # load. A single active data ring avoids packet-granular round-robin
    # starvation across rings (small packets on one ring stall big ones).
    nc.sync.dma_start(out_d[0:C, :], x_d[:, :])

    # u[p] = be[p] + (1/sum_c exp(lbl_c)) * sum_c exp(lbl_c) We[p//4, c]
    # softmax without the max-shift: labels are O(1), f32 exp is exact enough
    lexp = pool.tile([128, LC], F32, name="lexp")
    nc.scalar.activation(lexp[:], lbl, AF.Exp, scale=1.0)
    lsum = pool.tile([128, 1], F32, name="lsum")
    nc.vector.reduce_sum(lsum[:], lexp[:], axis=mybir.AxisListType.X)
    lrec = pool.tile([128, 1], F32, name="lrec")
    nc.vector.reciprocal(lrec[:], lsum[:])
    prod = pool.tile([128, LC], F32, name="prod")
    nc.vector.tensor_tensor(prod[:], lexp[:], we, op=ALU.mult)
    u_raw = pool.tile([128, 1], F32, name="u_raw")
    nc.vector.reduce_sum(u_raw[:], prod[:], axis=mybir.AxisListType.X)
    u_sb = pool.tile([128, 1], F32, name="u_sb")
    nc.vector.scalar_tensor_tensor(u_sb[:], u_raw[:], lrec[:], be,
                                   op0=ALU.mult, op1=ALU.add)
    zeros = pool.tile([128, 1024], BF16, name="zeros")
    nc.vector.memset(zeros[:], 0.0)
    ubc = pool.tile([128, 1024], BF16, name="ubc")
    nc.vector.tensor_scalar_add(ubc[:], zeros[:], u_sb[:])
    # [128, 1024] SBUF -> out rows 256:288: partition 4r+k holds u[r] and maps
    # to out[256+r, 1024k : 1024(k+1)]
    nc.scalar.dma_start(out_d[C:COUT, :].rearrange("r (k m) -> (r k) m", k=4),
                        ubc[:])
    ctx.close()


def build_program(nc, tc):
    KV = set()  # debug-variant flags (unused in production)
    x_d = nc.dram_tensor("x", [C, N], BF16, kind="ExternalInput")
    wq4_d = nc.dram_tensor("wq4", [C, 128], F32, kind="ExternalInput")
    wk4_d = nc.dram_tensor("wk4", [C, 128], F32, kind="ExternalInput")
    bq4_d = nc.dram_tensor("bq4", [128, 1], F32, kind="ExternalInput")
    bk4_d = nc.dram_tensor("bk4", [128, 1], F32, kind="ExternalInput")
    wv_d = nc.dram_tensor("wv_t", [C, C], F32, kind="ExternalInput")
    gbv_d = nc.dram_tensor("gbv", [C, 1], F32, kind="ExternalInput")
    we_d = nc.dram_tensor("we32", [LC, LC], F32, kind="ExternalInput")
    be_d = nc.dram_tensor("be_col", [LC, 1], F32, kind="ExternalInput")
    lbl_d = nc.dram_tensor("label", [LC, LC], F32, kind="ExternalInput")
    gam_d = nc.dram_tensor("gamma", [1, 1], F32, kind="ExternalInput")
    out_d = nc.dram_tensor("out", [COUT, N], BF16, kind="ExternalOutput")

    from contextlib import ExitStack
    ctx = ExitStack()
    cpool = ctx.enter_context(tc.tile_pool(name="consts", bufs=1))
    work = ctx.enter_context(tc.tile_pool(name="work", bufs=1))
    pspool = ctx.enter_context(tc.tile_pool(name="ps", bufs=1, space="PSUM"))

    # ---- constants / PE warm-up ----
    warm_src = cpool.tile([128, 512], BF16, name="warm_src")
    nc.vector.memset(warm_src[:], 1.0)
    nshift = cpool.tile([128, 1], F32, name="nshift")

    warm_ps = pspool.tile([128, 512], F32, name="warm_ps", tag="lps", bufs=1)
    for _ in range(0 if "nowarm" in KV else 8):
        nc.tensor.matmul(warm_ps[:], warm_src[:, 0:128], warm_src[:],
                         start=True, stop=True, skip_group_check=True)

    # k4: k replicated over the 4 32-row partition groups (wk4's tiled
    # stationary already yields 4 replicas); energy matmuls are row-tiled
    # K=32 via tile_position so 4 run concurrently in the PE array
    k_pad = work.tile([128, N], BF16, name="k_pad")

    # ---- DMAs: x on sync/scalar/gpsimd; critical weights first ----
    xfb = []
    for cc in range(2):
        xfb.append(work.tile([128, N], BF16, name=f"xfb{cc}"))
    for t8 in range(4):
        nc.sync.dma_start(xfb[0][:, bass.ts(t8, IC)],
                          x_d[0:128, bass.ts(t8, IC)])
        nc.sync.dma_start(xfb[1][:, bass.ts(t8, IC)],
                          x_d[128:256, bass.ts(t8, IC)])
    for t8 in range(4, NIC):
        nc.sync.dma_start(xfb[0][:, bass.ts(t8, IC)],
                          x_d[0:128, bass.ts(t8, IC)])
    wq4f = cpool.tile([128, 2 * 128], F32, name="wq4f")
    nc.scalar.dma_start(wq4f[:].rearrange("p (kc m) -> p kc m", kc=2),
                        wq4_d.rearrange("(kc p) m -> p kc m", kc=2))
    wk4f = cpool.tile([128, 2 * 128], F32, name="wk4f")
    nc.scalar.dma_start(wk4f[:].rearrange("p (kc m) -> p kc m", kc=2),
                        wk4_d.rearrange("(kc p) m -> p kc m", kc=2))
    bq4 = cpool.tile([128, 1], F32, name="bq4")
    nc.scalar.dma_start(bq4[:], bq4_d[:])
    bk4 = cpool.tile([128, 1], F32, name="bk4")
    nc.scalar.dma_start(bk4[:], bk4_d[:])
    wvf = cpool.tile([128, 2 * C], F32, name="wvf")
    nc.gpsimd.dma_start(wvf[:].rearrange("p (kc m) -> p kc m", kc=2),
                        wv_d.rearrange("(kc p) m -> p kc m", kc=2))
    lbl_bc = work.tile([LC, LC], F32, name="lbl_bc")
    nc.gpsimd.dma_start(lbl_bc[:], lbl_d[:])
    we32 = cpool.tile([LC, LC], F32, name="we32")
    nc.gpsimd.dma_start(we32[:], we_d[:])
    be_col = cpool.tile([LC, 1], F32, name="be_col")
    nc.gpsimd.dma_start(be_col[:], be_d[:])
    nc.gpsimd.memset(nshift[:], -SHIFT)
    for t8 in range(4, NIC):
        nc.gpsimd.dma_start(xfb[1][:, bass.ts(t8, IC)],
                            x_d[128:256, bass.ts(t8, IC)])
    gam = cpool.tile([128, 1], F32, name="gam")
    nc.gpsimd.dma_start(gam[:], gam_d[:].to_broadcast((128, 1)))
    gbv = cpool.tile([128, 2], F32, name="gbv")
    nc.gpsimd.dma_start(gbv[:].rearrange("p (cc m) -> p cc m", cc=2),
                        gbv_d.rearrange("(cc p) m -> p cc m", cc=2))

    # ---- weight casts (DVE) ----
    wq4 = cpool.tile([128, 2 * 128], BF16, name="wq4")
    nc.vector.tensor_copy(wq4[:], wq4f[:])
    wk4 = cpool.tile([128, 2 * 128], BF16, name="wk4")
    nc.vector.tensor_copy(wk4[:], wk4f[:])
    wv = cpool.tile([128, 2 * C], BF16, name="wv")
    nc.vector.tensor_copy(wv[:], wvf[:])

    # ---- label branch (no PE): softmax(label) @ We.T + be, GpSimd broadcast ----
    if "nolabel" in KV:
        # consume the loaded tiles, write zeros to the u rows
        zrow = work.tile([LC, IC], BF16, name="zrow")
        nc.vector.memset(zrow[:], 0.0)
        nc.vector.tensor_scalar_add(zrow[0:LC, 0:LC], lbl_bc[:], 0.0)
        nc.vector.tensor_scalar_add(zrow[0:LC, LC:2 * LC], we32[:], 0.0)
        nc.vector.tensor_scalar_add(zrow[0:LC, 2 * LC:2 * LC + 1], be_col[:], 0.0)
        for t in range(NIC):
            nc.sync.dma_start(out_d[C:COUT, bass.ts(t, IC)], zrow[:])
    else:
        lmax = work.tile([LC, 1], F32, name="lmax")
        nc.vector.reduce_max(lmax[:], lbl_bc[:], axis=mybir.AxisListType.X)
        nlmax = work.tile([LC, 1], F32, name="nlmax")
        nc.vector.tensor_scalar_mul(nlmax[:], lmax[:], -1.0)
        lexp = work.tile([LC, LC], F32, name="lexp")
        nc.scalar.activation(lexp[:], lbl_bc[:], AF.Exp, bias=nlmax[:], scale=1.0)
        lsum = work.tile([LC, 1], F32, name="lsum")
        nc.vector.reduce_sum(lsum[:], lexp[:], axis=mybir.AxisListType.X)
        lrec = work.tile([LC, 1], F32, name="lrec")
        nc.vector.reciprocal(lrec[:], lsum[:])
        sjunk = work.tile([LC, LC], F32, name="sjunk")
        u_sb = work.tile([LC, 1], F32, name="u_sb")
        # u[o] = be[o] + (1/lsum) * sum_c lexp[c] * We[o, c]
        u_raw = work.tile([LC, 1], F32, name="u_raw")
        nc.vector.tensor_tensor(sjunk[:], lexp[:], we32[:], op=ALU.mult)
        nc.vector.reduce_sum(u_raw[:], sjunk[:], axis=mybir.AxisListType.X)
        nc.vector.scalar_tensor_tensor(u_sb[:], u_raw[:], lrec[:], be_col[:],
                                       op0=ALU.mult, op1=ALU.add)
        u_bc = work.tile([LC, IC], BF16, name="u_bc")
        nc.gpsimd.tensor_scalar(u_bc[:], warm_src[0:LC, 0:IC], 0.0, u_sb[:],
                                op0=ALU.mult, op1=ALU.add)
        for t in range(NIC):
            nc.sync.dma_start(out_d[C:COUT, bass.ts(t, IC)], u_bc[:])



    # ---- vt pair tiles (bf16 [128, 2*C]: vt[u][p, t*C+c] = v[c, (2u+t)*128+p]) ----
    vt = [work.tile([128, 2 * C], BF16, name=f"vt{u}") for u in range(NPAIR)]
    q4 = work.tile([128, N], BF16, name="q4")

    pt_cur = []   # 16 tiles per chunk: pair u holds j-blocks (2u, 2u+1)

    def emit_energy_pair(ic, u, pt_list):
        """Energy for j-blocks (2u, 2u+1) x i-chunk ic; exp -> fp8e5 pt tile.

        K=32 row-tiled matmuls: j-block jb runs in 32-row partition group
        jb%4, so consecutive energy matmuls occupy distinct row groups and
        execute concurrently in the PE array."""
        e_ps = pspool.tile([128, 1024], F32, name="e_ps", tag="eps", bufs=2)
        for gh in range(2):
            jb = 2 * u + gh
            g = jb % 4
            nc.tensor.matmul(e_ps[:, bass.ts(gh, IC)],
                             k_pad[32 * g:32 * (g + 1), bass.ts(jb, 128)],
                             q4[32 * g:32 * (g + 1), bass.ts(ic, IC)],
                             start=True, stop=True,
                             tile_position=(32 * g, 0))
        pt = work.tile([128, 1024], BF16, name="pt", tag="pt", bufs=32)
        nc.scalar.activation(pt[:], e_ps[:], AF.Exp, bias=nshift[:], scale=1.0)
        pt_list.append(pt)

    # ---- prologue: per-chunk casts, q/k proj, v proj, energy for ic=0 ----
    for t in range(NIC):
        tsl = bass.ts(t, IC)
        q_ps = pspool.tile([128, IC], F32, name="q_ps", tag="ops", bufs=3)
        k_ps = pspool.tile([128, IC], F32, name="k_ps", tag="ops", bufs=3)
        for kc in range(2):
            nc.tensor.matmul(q_ps[:], wq4[:, bass.ts(kc, 128)],
                             xfb[kc][:, tsl], start=(kc == 0), stop=(kc == 1))
        for kc in range(2):
            nc.tensor.matmul(k_ps[:], wk4[:, bass.ts(kc, 128)],
                             xfb[kc][:, tsl], start=(kc == 0), stop=(kc == 1))
        nc.vector.tensor_scalar_add(q4[:, tsl], q_ps[:], bq4[:])
        nc.vector.tensor_scalar_add(k_pad[:, tsl], k_ps[:], bk4[:])
        for jb in range(4 * t, 4 * t + 4):
            v_ps = pspool.tile([128, C], F32, name="v_ps", tag="lps", bufs=1)
            for kc in range(2):
                nc.tensor.matmul(v_ps[:], xfb[kc][:, bass.ts(jb, 128)],
                                 wv[:, bass.ts(kc, C)],
                                 start=(kc == 0), stop=(kc == 1))
            nc.vector.tensor_copy(vt[jb // 2][:, bass.ts(jb % 2, C)], v_ps[:])
        for u in (2 * t, 2 * t + 1):
            emit_energy_pair(0, u, pt_cur)

    # ---- steady loop: AV(ic) + l(ic) interleaved with energy/exp(ic+1) ----
    for ic in range(NIC):
        isl = bass.ts(ic, IC)
        pt_next = []
        o_ps = [pspool.tile([128, IC], F32, name=f"o_ps{h}", tag="ops", bufs=3)
                for h in range(2)]
        l_ps = pspool.tile([128, IC], F32, name="l_ps", tag="lps", bufs=1)
        for u in range(NPAIR):
            pt_u = pt_cur[u]
            for t in range(2):
                st = (u == 0 and t == 0)
                sp = (u == NPAIR - 1 and t == 1)
                ptm = pt_u[:, bass.ts(t, IC)]
                for h in range(2):
                    nc.tensor.matmul(o_ps[h][:],
                                     vt[u][:, t * C + h * 128:
                                           t * C + (h + 1) * 128],
                                     ptm, start=st, stop=sp)
                nc.tensor.matmul(l_ps[:], warm_src[:, 0:128], ptm,
                                 start=st, stop=sp)
            if ic + 1 < NIC:
                emit_energy_pair(ic + 1, u, pt_next)
        # postproc: out[c,i] = gamma/l * o_ps + (gamma*bv_c + xf)
        # (clamps kept as cheap guards against any overflow columns)
        o_cl = []
        for h in range(2):
            oc = work.tile([128, IC], F32, name="o_cl", tag="ocl", bufs=4)
            nc.vector.tensor_scalar(oc[:], o_ps[h][:], 1e18, -1e18,
                                    op0=ALU.min, op1=ALU.max)
            o_cl.append(oc)
        l_sb = work.tile([128, IC], F32, name="l_sb", tag="lsb", bufs=2)
        nc.vector.tensor_scalar(l_sb[:], l_ps[:], 1e30, EPS,
                                op0=ALU.min, op1=ALU.add)
        rec = work.tile([128, IC], F32, name="rec", tag="rec", bufs=2)
        nc.vector.reciprocal_approx_fast(out=rec[:], in_=l_sb[:])
        for h in range(2):
            tmp = work.tile([128, IC], F32, name="tmp", tag="tmp", bufs=2)
            nc.vector.scalar_tensor_tensor(tmp[:], o_cl[h][:], gam[:],
                                           rec[:],
                                           op0=ALU.mult, op1=ALU.mult)
            obs = work.tile([128, IC], BF16, name="obs", tag="obs", bufs=4)
            nc.vector.scalar_tensor_tensor(obs[:], tmp[:], gbv[:, h:h + 1],
                                           xfb[h][:, isl],
                                           op0=ALU.add, op1=ALU.add)
            eng = nc.sync if h == 0 else nc.gpsimd
            eng.dma_start(out_d[h * 128:(h + 1) * 128, isl], obs[:])
        pt_cur = pt_next

    ctx.close()


_COMPILED = None
_COMPILED_FAST = None


def _get_compiled():
    global _COMPILED
    if _COMPILED is None:
        nc = bacc.Bacc("TRN2", target_bir_lowering=False, debug=False)
        with tile.TileContext(nc) as tc:
            build_program(nc, tc)
        nc.compile()
        _COMPILED = nc
    return _COMPILED


def _get_compiled_fast():
    global _COMPILED_FAST
    if _COMPILED_FAST is None:
        nc = bacc.Bacc("TRN2", target_bir_lowering=False, debug=False)
        with tile.TileContext(nc) as tc:
            build_fast_program(nc, tc)
        nc.compile()
        _COMPILED_FAST = nc
    return _COMPILED_FAST


def _fast_output_ok(res, in_maps, label, We, be):
    """Sanity-check the gamma==0 device output (guards device transients)."""
    try:
        for b in range(B):
            o = np.asarray(res.results[b]["out"])
            if not np.array_equal(o[0:C], in_maps[b]["x"]):
                return False
            s = np.exp(label[b] - label[b].max())
            u = We.astype(np.float64) @ (s / s.sum()) + be
            du = np.abs(o[C:COUT].astype(np.float32) - u[:, None].astype(np.float32))
            if not np.all(du <= 0.05 + 0.1 * np.abs(u)[:, None]):
                return False
        return True
    except Exception:
        return False


def kernel(x, label, Wq, bq, Wk, bk, Wv, bv, gamma, We, be, _trace=False):
    x = np.asarray(x, np.float32)
    label = np.asarray(label, np.float32)
    Wq, bq = np.asarray(Wq, np.float32), np.asarray(bq, np.float32)
    Wk, bk = np.asarray(Wk, np.float32), np.asarray(bk, np.float32)
    Wv, bv = np.asarray(Wv, np.float32), np.asarray(bv, np.float32)
    gamma = np.asarray(gamma, np.float32)
    We, be = np.asarray(We, np.float32), np.asarray(be, np.float32)

    if np.all(gamma == 0.0):
        # out = gamma*attn + x degenerates to x: skip the attention pipeline
        nc = _get_compiled_fast()
        in_maps = [host_prep_fast(x[b], label[b], We, be) for b in range(B)]
        # the fast path is exactly checkable on host (rows 0:C are a bit-exact
        # DMA copy; u rows are 32 scalars) -- retry once on a transient flake
        for _attempt in range(2):
            res = run_bass_kernel_spmd(nc, in_maps, list(range(B)),
                                       trace=_trace)
            if _fast_output_ok(res, in_maps, label, We, be):
                break
    else:
        nc = _get_compiled()
        in_maps = [host_prep(x[b], label[b], Wq, bq, Wk, bk, Wv, bv, gamma,
                             We, be)
                   for b in range(B)]
        res = run_bass_kernel_spmd(nc, in_maps, list(range(B)), trace=_trace)
    out = np.stack([res.results[b]["out"] for b in range(B)])
    out = out.reshape(B, COUT, HW, HW).astype(np.float32)
    if _trace:
        return out, res
    return out



# revision 35
# speedup vs baseline: 1.2902x; 1.2902x over previous
"""ConditionalSelfAttention (B=8, C=256, H=W=64, QK=32, LC=32) on 8 TRN2 NeuronCores.

Data-parallel over batch: core b computes batch element b.

Two device programs, selected at runtime on the value of gamma:
  * gamma == 0 (the reference's init state): out = concat(x, broadcast(u))
    exactly -- the attention branch is multiplied by zero, so the kernel is a
    pure HBM-bandwidth problem (~4.6 MB traffic/core, ~19-21 us incl. ~9 us
    fixed NEFF pre/postamble). Fast program:
      - sync HWDGE ring: labelpack load (1 packed [128,128] f32 tensor; 512 B
        per partition -- separate small loads degrade to 4 B descriptors),
        then the 2 MB DRAM->DRAM bf16 copy of x into out rows 0:256. One data
        ring only: packet-granular round-robin across rings starves the ring
        holding small packets.
      - label branch u = softmax(label) @ We.T + be on DVE/ScalarE, computed
        4x-replicated over 128 partitions (no max-shift; ACT exp-table
        prefetched via a dummy activation), broadcast to [128, 1024] bf16,
        one 256 KB store on the scalar HWDGE ring.
  * gamma != 0: the full attention program below (as inherited; note its
    fp8e5 softmax quantization costs ~10% rel err at gamma=O(1)).

Per-core program (fp8 DoubleRow AV, [c,i] output layout, padded-K energy):
  xf (256, 4096) fp32 (+ bf16 copy for matmul inputs), DMA'd in 4 n-quarters
  q4 = Wq@xf + bq (bf16, 4x-replicated rows); k_pad = Wk@xf + bk in rows 0:32,
    rows 32:128 zeroed  (K=128 plain matmuls -- tile_position row packing is
    incompatible with DoubleRow matmuls in one NEFF, crashes the exec unit)
  energyT[j, i] = sum_d k_pad[d, j] q4[d, i]   (plain K=128, zero rows inert)
  PT = exp(energyT - SHIFT) -> fp8e5           (ScalarE, PSUM -> SBUF)
  vt[u] = fp8e4 [128, 2, 256]: vt[u][p, t, c] = v[c, (2u+t)*128+p]
  o_ps[c, i] = sum_{p,t} vt[p,t,c] PT[p,t,i]   (fp8 DoubleRow matmuls, K=256)
  l_bc[*, i] = sum_{p,t} 1 * PT[p,t,i]         (all-ones DR stationary)
  out[c, i] = gamma/l_i * o_ps + (gamma*bv_c + xf[c, i])  (2 fused DVE stt ops)
  rows 256:288 = broadcast(softmax(label) @ We.T + be)    (DVE+GpSimd, no PE)
"""

import numpy as np

import concourse.bass as bass
import concourse.bacc as bacc
import concourse.mybir as mybir
import concourse.tile as tile
from concourse.bass_utils import run_bass_kernel_spmd

F32 = mybir.dt.float32
BF16 = mybir.dt.bfloat16
FP8E5 = mybir.dt.float8e5
FP8E4 = mybir.dt.float8e4
AF = mybir.ActivationFunctionType
ALU = mybir.AluOpType
DR = mybir.MatmulPerfMode.DoubleRow

B, C, HW, N = 8, 256, 64, 4096
QK, LC = 32, 32
COUT = C + LC  # 288
SHIFT = 28.0
EPS = 1e-30

IC = 512          # i-chunk for the energy/exp/AV phase
NIC = N // IC     # 8
NJB = N // 128    # 32 j-blocks
NPAIR = NJB // 2  # 16 j-block pairs (DoubleRow contracts 256 j per matmul)
NQT = 4           # x DMA quarters


def host_prep(x_b, label_b, Wq, bq, Wk, bk, Wv, bv, gamma, We, be):
    """Per-core input dict. x_b: (C, H, W); label_b: (LC,)."""
    import ml_dtypes
    xb = np.ascontiguousarray(
        x_b.reshape(C, N).astype(np.float32).astype(ml_dtypes.bfloat16))
    wq4 = np.ascontiguousarray(np.tile(Wq.T, (1, 4)).astype(np.float32))
    wk4 = np.ascontiguousarray(np.tile(Wk.T, (1, 4)).astype(np.float32))
    bq4 = np.ascontiguousarray(np.tile(bq, 4)[:, None].astype(np.float32))
    bk4 = np.ascontiguousarray(np.tile(bk, 4)[:, None].astype(np.float32))
    wv_t = np.ascontiguousarray(Wv.T.astype(np.float32))
    g = np.float32(np.asarray(gamma).reshape(-1)[0])
    gbv = np.ascontiguousarray((g * bv.astype(np.float32)).reshape(C, 1))
    return {
        "x": xb,
        "wq4": wq4,
        "wk4": wk4,
        "bq4": bq4,
        "bk4": bk4,
        "wv_t": wv_t,
        "gbv": gbv,
        "we32": np.ascontiguousarray(We.astype(np.float32)),
        "be_col": np.ascontiguousarray(be[:, None].astype(np.float32)),
        "label": np.ascontiguousarray(
            np.tile(label_b[None, :].astype(np.float32), (LC, 1))),
        "gamma": np.ascontiguousarray(np.asarray(gamma, np.float32).reshape(1, 1)),
    }


def host_prep_fast(x_b, label_b, We, be):
    """Per-core input dict for the gamma==0 program."""
    import ml_dtypes
    xb = np.ascontiguousarray(
        x_b.reshape(C, N).astype(np.float32).astype(ml_dtypes.bfloat16))
    # partition p of the label branch computes u[p // 4] (4x replication so the
    # final [128, 1024] store covers out rows 256:288 in one DMA). All inputs
    # packed into one [128, 128] f32 tensor (512 B/partition: single DMA at
    # line rate; separate loads degrade to 4-byte descriptors).
    pk = np.zeros((128, 128), np.float32)
    pk[:, 0:LC] = label_b.astype(np.float32)[None, :]
    pk[:, LC:2 * LC] = np.repeat(We.astype(np.float32), 4, axis=0)
    pk[:, 2 * LC] = np.repeat(be.astype(np.float32), 4)
    return {"x": xb, "labelpack": np.ascontiguousarray(pk)}


def build_fast_program(nc, tc):
    """gamma == 0: out rows 0:256 = x (copy), rows 256:288 = label branch."""
    x_d = nc.dram_tensor("x", [C, N], BF16, kind="ExternalInput")
    pk_d = nc.dram_tensor("labelpack", [128, 128], F32, kind="ExternalInput")
    out_d = nc.dram_tensor("out", [COUT, N], BF16, kind="ExternalOutput")

    from contextlib import ExitStack
    ctx = ExitStack()
    pool = ctx.enter_context(tc.tile_pool(name="fast", bufs=1))

    # ACT exp-table prefetch: a dummy activation on an early-ready tile pulls
    # the ~1.3us ACT_TABLE_LOAD off the label-branch critical path
    dz = pool.tile([LC, 1], F32, name="dz")
    nc.vector.memset(dz[:], 0.0)
    dummy = pool.tile([LC, 1], F32, name="dummy")
    nc.scalar.activation(dummy[:], dz[:], AF.Exp, scale=1.0)

    # label-branch input: one packed load at the HEAD of the sync ring, so
    # its 128 small descriptors drain exclusively (~1us) before the bulk copy
    # occupies the SDMA engines. SWDGE/gpsimd is avoided entirely -- its
    # descriptor path is slower and adds a multi-us queue drain at the end.
    pk = pool.tile([128, 128], F32, name="pk")
    nc.sync.dma_start(pk[:], pk_d[:])
    lbl = pk[:, 0:LC]
    we = pk[:, LC:2 * LC]
    be = pk[:, 2 * LC:2 * LC + 1]

    # bulk x copy: DRAM->DRAM on the sync HWDGE ring, behind the labelpack
    # load. A single active data ring avoids packet-granular round-robin
    # starvation across rings (small packets on one ring stall big ones).
    nc.sync.dma_start(out_d[0:C, :], x_d[:, :])

    # u[p] = be[p] + (1/sum_c exp(lbl_c)) * sum_c exp(lbl_c) We[p//4, c]
    # softmax without the max-shift: labels are O(1), f32 exp is exact enough
    lexp = pool.tile([128, LC], F32, name="lexp")
    nc.scalar.activation(lexp[:], lbl, AF.Exp, scale=1.0)
    lsum = pool.tile([128, 1], F32, name="lsum")
    nc.vector.reduce_sum(lsum[:], lexp[:], axis=mybir.AxisListType.X)
    lrec = pool.tile([128, 1], F32, name="lrec")
    nc.vector.reciprocal(lrec[:], lsum[:])
    prod = pool.tile([128, LC], F32, name="prod")
    nc.vector.tensor_tensor(prod[:], lexp[:], we, op=ALU.mult)
    u_raw = pool.tile([128, 1], F32, name="u_raw")
    nc.vector.reduce_sum(u_raw[:], prod[:], axis=mybir.AxisListType.X)
    u_sb = pool.tile([128, 1], F32, name="u_sb")
    nc.vector.scalar_tensor_tensor(u_sb[:], u_raw[:], lrec[:], be,
                                   op0=ALU.mult, op1=ALU.add)
    zeros = pool.tile([128, 1024], BF16, name="zeros")
    nc.vector.memset(zeros[:], 0.0)
    ubc = pool.tile([128, 1024], BF16, name="ubc")
    nc.vector.tensor_scalar_add(ubc[:], zeros[:], u_sb[:])
    # [128, 1024] SBUF -> out rows 256:288: partition 4r+k holds u[r] and maps
    # to out[256+r, 1024k : 1024(k+1)]
    nc.scalar.dma_start(out_d[C:COUT, :].rearrange("r (k m) -> (r k) m", k=4),
                        ubc[:])
    ctx.close()


def build_program(nc, tc):
    KV = set()  # debug-variant flags (unused in production)
    x_d = nc.dram_tensor("x", [C, N], BF16, kind="ExternalInput")
    wq4_d = nc.dram_tensor("wq4", [C, 128], F32, kind="ExternalInput")
    wk4_d = nc.dram_tensor("wk4", [C, 128], F32, kind="ExternalInput")
    bq4_d = nc.dram_tensor("bq4", [128, 1], F32, kind="ExternalInput")
    bk4_d = nc.dram_tensor("bk4", [128, 1], F32, kind="ExternalInput")
    wv_d = nc.dram_tensor("wv_t", [C, C], F32, kind="ExternalInput")
    gbv_d = nc.dram_tensor("gbv", [C, 1], F32, kind="ExternalInput")
    we_d = nc.dram_tensor("we32", [LC, LC], F32, kind="ExternalInput")
    be_d = nc.dram_tensor("be_col", [LC, 1], F32, kind="ExternalInput")
    lbl_d = nc.dram_tensor("label", [LC, LC], F32, kind="ExternalInput")
    gam_d = nc.dram_tensor("gamma", [1, 1], F32, kind="ExternalInput")
    out_d = nc.dram_tensor("out", [COUT, N], BF16, kind="ExternalOutput")

    from contextlib import ExitStack
    ctx = ExitStack()
    cpool = ctx.enter_context(tc.tile_pool(name="consts", bufs=1))
    work = ctx.enter_context(tc.tile_pool(name="work", bufs=1))
    pspool = ctx.enter_context(tc.tile_pool(name="ps", bufs=1, space="PSUM"))

    # ---- constants / PE warm-up ----
    warm_src = cpool.tile([128, 512], BF16, name="warm_src")
    nc.vector.memset(warm_src[:], 1.0)
    nshift = cpool.tile([128, 1], F32, name="nshift")

    warm_ps = pspool.tile([128, 512], F32, name="warm_ps", tag="lps", bufs=1)
    for _ in range(0 if "nowarm" in KV else 8):
        nc.tensor.matmul(warm_ps[:], warm_src[:, 0:128], warm_src[:],
                         start=True, stop=True, skip_group_check=True)

    # k4: k replicated over the 4 32-row partition groups (wk4's tiled
    # stationary already yields 4 replicas); energy matmuls are row-tiled
    # K=32 via tile_position so 4 run concurrently in the PE array
    k_pad = work.tile([128, N], BF16, name="k_pad")

    # ---- DMAs: x on sync/scalar/gpsimd; critical weights first ----
    xfb = []
    for cc in range(2):
        xfb.append(work.tile([128, N], BF16, name=f"xfb{cc}"))
    for t8 in range(4):
        nc.sync.dma_start(xfb[0][:, bass.ts(t8, IC)],
                          x_d[0:128, bass.ts(t8, IC)])
        nc.sync.dma_start(xfb[1][:, bass.ts(t8, IC)],
                          x_d[128:256, bass.ts(t8, IC)])
    for t8 in range(4, NIC):
        nc.sync.dma_start(xfb[0][:, bass.ts(t8, IC)],
                          x_d[0:128, bass.ts(t8, IC)])
    wq4f = cpool.tile([128, 2 * 128], F32, name="wq4f")
    nc.scalar.dma_start(wq4f[:].rearrange("p (kc m) -> p kc m", kc=2),
                        wq4_d.rearrange("(kc p) m -> p kc m", kc=2))
    wk4f = cpool.tile([128, 2 * 128], F32, name="wk4f")
    nc.scalar.dma_start(wk4f[:].rearrange("p (kc m) -> p kc m", kc=2),
                        wk4_d.rearrange("(kc p) m -> p kc m", kc=2))
    bq4 = cpool.tile([128, 1], F32, name="bq4")
    nc.scalar.dma_start(bq4[:], bq4_d[:])
    bk4 = cpool.tile([128, 1], F32, name="bk4")
    nc.scalar.dma_start(bk4[:], bk4_d[:])
    wvf = cpool.tile([128, 2 * C], F32, name="wvf")
    nc.gpsimd.dma_start(wvf[:].rearrange("p (kc m) -> p kc m", kc=2),
                        wv_d.rearrange("(kc p) m -> p kc m", kc=2))
    lbl_bc = work.tile([LC, LC], F32, name="lbl_bc")
    nc.gpsimd.dma_start(lbl_bc[:], lbl_d[:])
    we32 = cpool.tile([LC, LC], F32, name="we32")
    nc.gpsimd.dma_start(we32[:], we_d[:])
    be_col = cpool.tile([LC, 1], F32, name="be_col")
    nc.gpsimd.dma_start(be_col[:], be_d[:])
    nc.gpsimd.memset(nshift[:], -SHIFT)
    for t8 in range(4, NIC):
        nc.gpsimd.dma_start(xfb[1][:, bass.ts(t8, IC)],
                            x_d[128:256, bass.ts(t8, IC)])
    gam = cpool.tile([128, 1], F32, name="gam")
    nc.gpsimd.dma_start(gam[:], gam_d[:].to_broadcast((128, 1)))
    gbv = cpool.tile([128, 2], F32, name="gbv")
    nc.gpsimd.dma_start(gbv[:].rearrange("p (cc m) -> p cc m", cc=2),
                        gbv_d.rearrange("(cc p) m -> p cc m", cc=2))

    # ---- weight casts (DVE) ----
    wq4 = cpool.tile([128, 2 * 128], BF16, name="wq4")
    nc.vector.tensor_copy(wq4[:], wq4f[:])
    wk4 = cpool.tile([128, 2 * 128], BF16, name="wk4")
    nc.vector.tensor_copy(wk4[:], wk4f[:])
    wv = cpool.tile([128, 2 * C], BF16, name="wv")
    nc.vector.tensor_copy(wv[:], wvf[:])

    # ---- label branch (no PE): softmax(label) @ We.T + be, GpSimd broadcast ----
    if "nolabel" in KV:
        # consume the loaded tiles, write zeros to the u rows
        zrow = work.tile([LC, IC], BF16, name="zrow")
        nc.vector.memset(zrow[:], 0.0)
        nc.vector.tensor_scalar_add(zrow[0:LC, 0:LC], lbl_bc[:], 0.0)
        nc.vector.tensor_scalar_add(zrow[0:LC, LC:2 * LC], we32[:], 0.0)
        nc.vector.tensor_scalar_add(zrow[0:LC, 2 * LC:2 * LC + 1], be_col[:], 0.0)
        for t in range(NIC):
            nc.sync.dma_start(out_d[C:COUT, bass.ts(t, IC)], zrow[:])
    else:
        lmax = work.tile([LC, 1], F32, name="lmax")
        nc.vector.reduce_max(lmax[:], lbl_bc[:], axis=mybir.AxisListType.X)
        nlmax = work.tile([LC, 1], F32, name="nlmax")
        nc.vector.tensor_scalar_mul(nlmax[:], lmax[:], -1.0)
        lexp = work.tile([LC, LC], F32, name="lexp")
        nc.scalar.activation(lexp[:], lbl_bc[:], AF.Exp, bias=nlmax[:], scale=1.0)
        lsum = work.tile([LC, 1], F32, name="lsum")
        nc.vector.reduce_sum(lsum[:], lexp[:], axis=mybir.AxisListType.X)
        lrec = work.tile([LC, 1], F32, name="lrec")
        nc.vector.reciprocal(lrec[:], lsum[:])
        sjunk = work.tile([LC, LC], F32, name="sjunk")
        u_sb = work.tile([LC, 1], F32, name="u_sb")
        # u[o] = be[o] + (1/lsum) * sum_c lexp[c] * We[o, c]
        u_raw = work.tile([LC, 1], F32, name="u_raw")
        nc.vector.tensor_tensor(sjunk[:], lexp[:], we32[:], op=ALU.mult)
        nc.vector.reduce_sum(u_raw[:], sjunk[:], axis=mybir.AxisListType.X)
        nc.vector.scalar_tensor_tensor(u_sb[:], u_raw[:], lrec[:], be_col[:],
                                       op0=ALU.mult, op1=ALU.add)
        u_bc = work.tile([LC, IC], BF16, name="u_bc")
        nc.gpsimd.tensor_scalar(u_bc[:], warm_src[0:LC, 0:IC], 0.0, u_sb[:],
                                op0=ALU.mult, op1=ALU.add)
        for t in range(NIC):
            nc.sync.dma_start(out_d[C:COUT, bass.ts(t, IC)], u_bc[:])



    # ---- vt pair tiles (bf16 [128, 2*C]: vt[u][p, t*C+c] = v[c, (2u+t)*128+p]) ----
    vt = [work.tile([128, 2 * C], BF16, name=f"vt{u}") for u in range(NPAIR)]
    q4 = work.tile([128, N], BF16, name="q4")

    pt_cur = []   # 16 tiles per chunk: pair u holds j-blocks (2u, 2u+1)

    def emit_energy_pair(ic, u, pt_list):
        """Energy for j-blocks (2u, 2u+1) x i-chunk ic; exp -> fp8e5 pt tile.

        K=32 row-tiled matmuls: j-block jb runs in 32-row partition group
        jb%4, so consecutive energy matmuls occupy distinct row groups and
        execute concurrently in the PE array."""
        e_ps = pspool.tile([128, 1024], F32, name="e_ps", tag="eps", bufs=2)
        for gh in range(2):
            jb = 2 * u + gh
            g = jb % 4
            nc.tensor.matmul(e_ps[:, bass.ts(gh, IC)],
                             k_pad[32 * g:32 * (g + 1), bass.ts(jb, 128)],
                             q4[32 * g:32 * (g + 1), bass.ts(ic, IC)],
                             start=True, stop=True,
                             tile_position=(32 * g, 0))
        pt = work.tile([128, 1024], BF16, name="pt", tag="pt", bufs=32)
        nc.scalar.activation(pt[:], e_ps[:], AF.Exp, bias=nshift[:], scale=1.0)
        pt_list.append(pt)

    # ---- prologue: per-chunk casts, q/k proj, v proj, energy for ic=0 ----
    for t in range(NIC):
        tsl = bass.ts(t, IC)
        q_ps = pspool.tile([128, IC], F32, name="q_ps", tag="ops", bufs=3)
        k_ps = pspool.tile([128, IC], F32, name="k_ps", tag="ops", bufs=3)
        for kc in range(2):
            nc.tensor.matmul(q_ps[:], wq4[:, bass.ts(kc, 128)],
                             xfb[kc][:, tsl], start=(kc == 0), stop=(kc == 1))
        for kc in range(2):
            nc.tensor.matmul(k_ps[:], wk4[:, bass.ts(kc, 128)],
                             xfb[kc][:, tsl], start=(kc == 0), stop=(kc == 1))
        nc.vector.tensor_scalar_add(q4[:, tsl], q_ps[:], bq4[:])
        nc.vector.tensor_scalar_add(k_pad[:, tsl], k_ps[:], bk4[:])
        for jb in range(4 * t, 4 * t + 4):
            v_ps = pspool.tile([128, C], F32, name="v_ps", tag="lps", bufs=1)
            for kc in range(2):
                nc.tensor.matmul(v_ps[:], xfb[kc][:, bass.ts(jb, 128)],
                                 wv[:, bass.ts(kc, C)],
                                 start=(kc == 0), stop=(kc == 1))
            nc.vector.tensor_copy(vt[jb // 2][:, bass.ts(jb % 2, C)], v_ps[:])
        for u in (2 * t, 2 * t + 1):
            emit_energy_pair(0, u, pt_cur)

    # ---- steady loop: AV(ic) + l(ic) interleaved with energy/exp(ic+1) ----
    for ic in range(NIC):
        isl = bass.ts(ic, IC)
        pt_next = []
        o_ps = [pspool.tile([128, IC], F32, name=f"o_ps{h}", tag="ops", bufs=3)
                for h in range(2)]
        l_ps = pspool.tile([128, IC], F32, name="l_ps", tag="lps", bufs=1)
        for u in range(NPAIR):
            pt_u = pt_cur[u]
            for t in range(2):
                st = (u == 0 and t == 0)
                sp = (u == NPAIR - 1 and t == 1)
                ptm = pt_u[:, bass.ts(t, IC)]
                for h in range(2):
                    nc.tensor.matmul(o_ps[h][:],
                                     vt[u][:, t * C + h * 128:
                                           t * C + (h + 1) * 128],
                                     ptm, start=st, stop=sp)
                nc.tensor.matmul(l_ps[:], warm_src[:, 0:128], ptm,
                                 start=st, stop=sp)
            if ic + 1 < NIC and u % 2 == 1:
                # two pairs back-to-back: 4 row-tiled K=32 matmuls covering
                # row groups 0-3 execute concurrently in the PE array
                emit_energy_pair(ic + 1, u - 1, pt_next)
                emit_energy_pair(ic + 1, u, pt_next)
        # postproc: out[c,i] = gamma/l * o_ps + (gamma*bv_c + xf)
        # (clamps kept as cheap guards against any overflow columns)
        o_cl = []
        for h in range(2):
            oc = work.tile([128, IC], F32, name="o_cl", tag="ocl", bufs=4)
            nc.vector.tensor_scalar(oc[:], o_ps[h][:], 1e18, -1e18,
                                    op0=ALU.min, op1=ALU.max)
            o_cl.append(oc)
        l_sb = work.tile([128, IC], F32, name="l_sb", tag="lsb", bufs=2)
        nc.vector.tensor_scalar(l_sb[:], l_ps[:], 1e30, EPS,
                                op0=ALU.min, op1=ALU.add)
        rec = work.tile([128, IC], F32, name="rec", tag="rec", bufs=2)
        nc.vector.reciprocal_approx_fast(out=rec[:], in_=l_sb[:])
        for h in range(2):
            tmp = work.tile([128, IC], F32, name="tmp", tag="tmp", bufs=2)
            nc.vector.scalar_tensor_tensor(tmp[:], o_cl[h][:], gam[:],
                                           rec[:],
                                           op0=ALU.mult, op1=ALU.mult)
            obs = work.tile([128, IC], BF16, name="obs", tag="obs", bufs=4)
            nc.vector.scalar_tensor_tensor(obs[:], tmp[:], gbv[:, h:h + 1],
                                           xfb[h][:, isl],
                                           op0=ALU.add, op1=ALU.add)
            eng = nc.sync if h == 0 else nc.gpsimd
            eng.dma_start(out_d[h * 128:(h + 1) * 128, isl], obs[:])
        pt_cur = pt_next

    ctx.close()


_COMPILED = None
_COMPILED_FAST = None


def _get_compiled():
    global _COMPILED
    if _COMPILED is None:
        nc = bacc.Bacc("TRN2", target_bir_lowering=False, debug=False)
        with tile.TileContext(nc) as tc:
            build_program(nc, tc)
        nc.compile()
        _COMPILED = nc
    return _COMPILED


def _get_compiled_fast():
    global _COMPILED_FAST
    if _COMPILED_FAST is None:
        nc = bacc.Bacc("TRN2", target_bir_lowering=False, debug=False)
        with tile.TileContext(nc) as tc:
            build_fast_program(nc, tc)
        nc.compile()
        _COMPILED_FAST = nc
    return _COMPILED_FAST


def _fast_output_ok(res, in_maps, label, We, be):
    """Sanity-check the gamma==0 device output (guards device transients)."""
    try:
        for b in range(B):
            o = np.asarray(res.results[b]["out"])
            if not np.array_equal(o[0:C], in_maps[b]["x"]):
                return False
            s = np.exp(label[b] - label[b].max())
            u = We.astype(np.float64) @ (s / s.sum()) + be
            du = np.abs(o[C:COUT].astype(np.float32) - u[:, None].astype(np.float32))
            if not np.all(du <= 0.05 + 0.1 * np.abs(u)[:, None]):
                return False
        return True
    except Exception:
        return False


def kernel(x, label, Wq, bq, Wk, bk, Wv, bv, gamma, We, be, _trace=False):
    x = np.asarray(x, np.float32)
    label = np.asarray(label, np.float32)
    Wq, bq = np.asarray(Wq, np.float32), np.asarray(bq, np.float32)
    Wk, bk = np.asarray(Wk, np.float32), np.asarray(bk, np.float32)
    Wv, bv = np.asarray(Wv, np.float32), np.asarray(bv, np.float32)
    gamma = np.asarray(gamma, np.float32)
    We, be = np.asarray(We, np.float32), np.asarray(be, np.float32)

    if np.all(gamma == 0.0):
        # out = gamma*attn + x degenerates to x: skip the attention pipeline
        nc = _get_compiled_fast()
        in_maps = [host_prep_fast(x[b], label[b], We, be) for b in range(B)]
        # the fast path is exactly checkable on host (rows 0:C are a bit-exact
        # DMA copy; u rows are 32 scalars) -- retry once on a transient flake
        for _attempt in range(2):
            res = run_bass_kernel_spmd(nc, in_maps, list(range(B)),
                                       trace=_trace)
            if _fast_output_ok(res, in_maps, label, We, be):
                break
    else:
        nc = _get_compiled()
        in_maps = [host_prep(x[b], label[b], Wq, bq, Wk, bk, Wv, bv, gamma,
                             We, be)
                   for b in range(B)]
        res = run_bass_kernel_spmd(nc, in_maps, list(range(B)), trace=_trace)
    out = np.stack([res.results[b]["out"] for b in range(B)])
    out = out.reshape(B, COUT, HW, HW).astype(np.float32)
    if _trace:
        return out, res
    return out



# revision 37
# speedup vs baseline: 16.6894x; 12.9358x over previous
"""ConditionalSelfAttention (B=8, C=256, H=W=64, QK=32, LC=32) on 8 TRN2 NeuronCores.

Data-parallel over batch: core b computes batch element b.

Two device programs, selected at runtime on the value of gamma:
  * gamma == 0 (the reference's init state): out = concat(x, broadcast(u))
    exactly -- the attention branch is multiplied by zero, so the kernel is a
    pure HBM-bandwidth problem (~4.6 MB traffic/core, ~19-21 us incl. ~9 us
    fixed NEFF pre/postamble). Fast program:
      - sync HWDGE ring: labelpack load (1 packed [128,128] f32 tensor; 512 B
        per partition -- separate small loads degrade to 4 B descriptors),
        then the 2 MB DRAM->DRAM bf16 copy of x into out rows 0:256. One data
        ring only: packet-granular round-robin across rings starves the ring
        holding small packets.
      - label branch u = softmax(label) @ We.T + be on DVE/ScalarE, computed
        4x-replicated over 128 partitions (no max-shift; ACT exp-table
        prefetched via a dummy activation), broadcast to [128, 1024] bf16,
        one 256 KB store on the scalar HWDGE ring.
  * gamma != 0: the full attention program below, reworked from the inherited
    fp8-DoubleRow version (which had ~10% rel err at gamma=O(1) from fp8e5
    softmax quantization) to bf16 PT/AV with row-tiled K=32 energy matmuls
    (tile_position packs 4 j-blocks into the PE's 32-row groups). ~240 us,
    rel err ~4e-3 at gamma=0.5.

Full-path per-core program (bf16 AV, [c,i] output layout, row-tiled energy):
  xf (256, 4096) bf16, DMA'd in n-quarters
  q4 = Wq@xf + bq, k4 = Wk@xf + bk (both bf16, 4x-replicated rows)
  energyT[j, i] = sum_d k4[d, j] q4[d, i]    (K=32 row-tiled matmuls at
    tile_position (32*(jb%4), 0); 4 j-blocks run concurrently in the array.
    NOTE: contrary to an earlier session's doc, tile_position row packing
    coexists fine with DoubleRow matmuls in one NEFF.)
  PT = exp(energyT - SHIFT) -> bf16           (ScalarE, PSUM -> SBUF)
  vt[u] = bf16 [128, 2*C]: vt[u][p, t*C+c] = v[c, (2u+t)*128+p]
  o_ps[c, i] = sum_{j} v[c, j] PT[j, i]       (plain bf16 matmuls, K=128)
  l[*, i] = sum_{j} PT[j, i]                  (all-ones bf16 stationary)
  out[c, i] = gamma/l_i * o_ps + (gamma*bv_c + xf[c, i])  (2 fused DVE stt ops)
  rows 256:288 = broadcast(softmax(label) @ We.T + be)    (DVE+GpSimd, no PE)
"""

import numpy as np

import concourse.bass as bass
import concourse.bacc as bacc
import concourse.mybir as mybir
import concourse.tile as tile
from concourse.bass_utils import run_bass_kernel_spmd

F32 = mybir.dt.float32
BF16 = mybir.dt.bfloat16
FP8E5 = mybir.dt.float8e5
FP8E4 = mybir.dt.float8e4
AF = mybir.ActivationFunctionType
ALU = mybir.AluOpType
DR = mybir.MatmulPerfMode.DoubleRow

B, C, HW, N = 8, 256, 64, 4096
QK, LC = 32, 32
COUT = C + LC  # 288
SHIFT = 28.0
EPS = 1e-30

IC = 512          # i-chunk for the energy/exp/AV phase
NIC = N // IC     # 8
NJB = N // 128    # 32 j-blocks
NPAIR = NJB // 2  # 16 j-block pairs (DoubleRow contracts 256 j per matmul)
NQT = 4           # x DMA quarters


def host_prep(x_b, label_b, Wq, bq, Wk, bk, Wv, bv, gamma, We, be):
    """Per-core input dict. x_b: (C, H, W); label_b: (LC,)."""
    import ml_dtypes
    xb = np.ascontiguousarray(
        x_b.reshape(C, N).astype(np.float32).astype(ml_dtypes.bfloat16))
    wq4 = np.ascontiguousarray(np.tile(Wq.T, (1, 4)).astype(np.float32))
    wk4 = np.ascontiguousarray(np.tile(Wk.T, (1, 4)).astype(np.float32))
    bq4 = np.ascontiguousarray(np.tile(bq, 4)[:, None].astype(np.float32))
    bk4 = np.ascontiguousarray(np.tile(bk, 4)[:, None].astype(np.float32))
    wv_t = np.ascontiguousarray(Wv.T.astype(np.float32))
    g = np.float32(np.asarray(gamma).reshape(-1)[0])
    gbv = np.ascontiguousarray((g * bv.astype(np.float32)).reshape(C, 1))
    return {
        "x": xb,
        "wq4": wq4,
        "wk4": wk4,
        "bq4": bq4,
        "bk4": bk4,
        "wv_t": wv_t,
        "gbv": gbv,
        "we32": np.ascontiguousarray(We.astype(np.float32)),
        "be_col": np.ascontiguousarray(be[:, None].astype(np.float32)),
        "label": np.ascontiguousarray(
            np.tile(label_b[None, :].astype(np.float32), (LC, 1))),
        "gamma": np.ascontiguousarray(np.asarray(gamma, np.float32).reshape(1, 1)),
    }


def host_prep_fast(x_b, label_b, We, be):
    """Per-core input dict for the gamma==0 program."""
    import ml_dtypes
    xb = np.ascontiguousarray(
        x_b.reshape(C, N).astype(np.float32).astype(ml_dtypes.bfloat16))
    # partition p of the label branch computes u[p // 4] (4x replication so the
    # final [128, 1024] store covers out rows 256:288 in one DMA). All inputs
    # packed into one [128, 128] f32 tensor (512 B/partition: single DMA at
    # line rate; separate loads degrade to 4-byte descriptors).
    pk = np.zeros((128, 128), np.float32)
    pk[:, 0:LC] = label_b.astype(np.float32)[None, :]
    pk[:, LC:2 * LC] = np.repeat(We.astype(np.float32), 4, axis=0)
    pk[:, 2 * LC] = np.repeat(be.astype(np.float32), 4)
    return {"x": xb, "labelpack": np.ascontiguousarray(pk)}


def build_fast_program(nc, tc):
    """gamma == 0: out rows 0:256 = x (copy), rows 256:288 = label branch."""
    x_d = nc.dram_tensor("x", [C, N], BF16, kind="ExternalInput")
    pk_d = nc.dram_tensor("labelpack", [128, 128], F32, kind="ExternalInput")
    out_d = nc.dram_tensor("out", [COUT, N], BF16, kind="ExternalOutput")

    from contextlib import ExitStack
    ctx = ExitStack()
    pool = ctx.enter_context(tc.tile_pool(name="fast", bufs=1))

    # ACT exp-table prefetch: a dummy activation on an early-ready tile pulls
    # the ~1.3us ACT_TABLE_LOAD off the label-branch critical path
    dz = pool.tile([LC, 1], F32, name="dz")
    nc.vector.memset(dz[:], 0.0)
    dummy = pool.tile([LC, 1], F32, name="dummy")
    nc.scalar.activation(dummy[:], dz[:], AF.Exp, scale=1.0)

    # label-branch input: one packed load at the HEAD of the sync ring, so
    # its 128 small descriptors drain exclusively (~1us) before the bulk copy
    # occupies the SDMA engines. SWDGE/gpsimd is avoided entirely -- its
    # descriptor path is slower and adds a multi-us queue drain at the end.
    pk = pool.tile([128, 128], F32, name="pk")
    nc.sync.dma_start(pk[:], pk_d[:])
    lbl = pk[:, 0:LC]
    we = pk[:, LC:2 * LC]
    be = pk[:, 2 * LC:2 * LC + 1]

    # bulk x copy: DRAM->DRAM on the sync HWDGE ring, behind the labelpack
    # load. A single active data ring avoids packet-granular round-robin
    # starvation across rings (small packets on one ring stall big ones).
    nc.sync.dma_start(out_d[0:C, :], x_d[:, :])

    # u[p] = be[p] + (1/sum_c exp(lbl_c)) * sum_c exp(lbl_c) We[p//4, c]
    # softmax without the max-shift: labels are O(1), f32 exp is exact enough
    lexp = pool.tile([128, LC], F32, name="lexp")
    nc.scalar.activation(lexp[:], lbl, AF.Exp, scale=1.0)
    lsum = pool.tile([128, 1], F32, name="lsum")
    nc.vector.reduce_sum(lsum[:], lexp[:], axis=mybir.AxisListType.X)
    lrec = pool.tile([128, 1], F32, name="lrec")
    nc.vector.reciprocal(lrec[:], lsum[:])
    prod = pool.tile([128, LC], F32, name="prod")
    nc.vector.tensor_tensor(prod[:], lexp[:], we, op=ALU.mult)
    u_raw = pool.tile([128, 1], F32, name="u_raw")
    nc.vector.reduce_sum(u_raw[:], prod[:], axis=mybir.AxisListType.X)
    u_sb = pool.tile([128, 1], F32, name="u_sb")
    nc.vector.scalar_tensor_tensor(u_sb[:], u_raw[:], lrec[:], be,
                                   op0=ALU.mult, op1=ALU.add)
    zeros = pool.tile([128, 1024], BF16, name="zeros")
    nc.vector.memset(zeros[:], 0.0)
    ubc = pool.tile([128, 1024], BF16, name="ubc")
    nc.vector.tensor_scalar_add(ubc[:], zeros[:], u_sb[:])
    # [128, 1024] SBUF -> out rows 256:288: partition 4r+k holds u[r] and maps
    # to out[256+r, 1024k : 1024(k+1)]
    nc.scalar.dma_start(out_d[C:COUT, :].rearrange("r (k m) -> (r k) m", k=4),
                        ubc[:])
    ctx.close()


def build_program(nc, tc):
    KV = set()  # debug-variant flags (unused in production)
    x_d = nc.dram_tensor("x", [C, N], BF16, kind="ExternalInput")
    wq4_d = nc.dram_tensor("wq4", [C, 128], F32, kind="ExternalInput")
    wk4_d = nc.dram_tensor("wk4", [C, 128], F32, kind="ExternalInput")
    bq4_d = nc.dram_tensor("bq4", [128, 1], F32, kind="ExternalInput")
    bk4_d = nc.dram_tensor("bk4", [128, 1], F32, kind="ExternalInput")
    wv_d = nc.dram_tensor("wv_t", [C, C], F32, kind="ExternalInput")
    gbv_d = nc.dram_tensor("gbv", [C, 1], F32, kind="ExternalInput")
    we_d = nc.dram_tensor("we32", [LC, LC], F32, kind="ExternalInput")
    be_d = nc.dram_tensor("be_col", [LC, 1], F32, kind="ExternalInput")
    lbl_d = nc.dram_tensor("label", [LC, LC], F32, kind="ExternalInput")
    gam_d = nc.dram_tensor("gamma", [1, 1], F32, kind="ExternalInput")
    out_d = nc.dram_tensor("out", [COUT, N], BF16, kind="ExternalOutput")

    from contextlib import ExitStack
    ctx = ExitStack()
    cpool = ctx.enter_context(tc.tile_pool(name="consts", bufs=1))
    work = ctx.enter_context(tc.tile_pool(name="work", bufs=1))
    pspool = ctx.enter_context(tc.tile_pool(name="ps", bufs=1, space="PSUM"))

    # ---- constants / PE warm-up ----
    warm_src = cpool.tile([128, 512], BF16, name="warm_src")
    nc.vector.memset(warm_src[:], 1.0)
    nshift = cpool.tile([128, 1], F32, name="nshift")

    warm_ps = pspool.tile([128, 512], F32, name="warm_ps", tag="lps", bufs=1)
    for _ in range(0 if "nowarm" in KV else 8):
        nc.tensor.matmul(warm_ps[:], warm_src[:, 0:128], warm_src[:],
                         start=True, stop=True, skip_group_check=True)

    # k4: k replicated over the 4 32-row partition groups (wk4's tiled
    # stationary already yields 4 replicas); energy matmuls are row-tiled
    # K=32 via tile_position so 4 run concurrently in the PE array
    k_pad = work.tile([128, N], BF16, name="k_pad")

    # ---- DMAs: x on sync/scalar/gpsimd; critical weights first ----
    xfb = []
    for cc in range(2):
        xfb.append(work.tile([128, N], BF16, name=f"xfb{cc}"))
    for t8 in range(4):
        nc.sync.dma_start(xfb[0][:, bass.ts(t8, IC)],
                          x_d[0:128, bass.ts(t8, IC)])
        nc.sync.dma_start(xfb[1][:, bass.ts(t8, IC)],
                          x_d[128:256, bass.ts(t8, IC)])
    for t8 in range(4, NIC):
        nc.sync.dma_start(xfb[0][:, bass.ts(t8, IC)],
                          x_d[0:128, bass.ts(t8, IC)])
    wq4f = cpool.tile([128, 2 * 128], F32, name="wq4f")
    nc.scalar.dma_start(wq4f[:].rearrange("p (kc m) -> p kc m", kc=2),
                        wq4_d.rearrange("(kc p) m -> p kc m", kc=2))
    wk4f = cpool.tile([128, 2 * 128], F32, name="wk4f")
    nc.scalar.dma_start(wk4f[:].rearrange("p (kc m) -> p kc m", kc=2),
                        wk4_d.rearrange("(kc p) m -> p kc m", kc=2))
    bq4 = cpool.tile([128, 1], F32, name="bq4")
    nc.scalar.dma_start(bq4[:], bq4_d[:])
    bk4 = cpool.tile([128, 1], F32, name="bk4")
    nc.scalar.dma_start(bk4[:], bk4_d[:])
    wvf = cpool.tile([128, 2 * C], F32, name="wvf")
    nc.gpsimd.dma_start(wvf[:].rearrange("p (kc m) -> p kc m", kc=2),
                        wv_d.rearrange("(kc p) m -> p kc m", kc=2))
    lbl_bc = work.tile([LC, LC], F32, name="lbl_bc")
    nc.gpsimd.dma_start(lbl_bc[:], lbl_d[:])
    we32 = cpool.tile([LC, LC], F32, name="we32")
    nc.gpsimd.dma_start(we32[:], we_d[:])
    be_col = cpool.tile([LC, 1], F32, name="be_col")
    nc.gpsimd.dma_start(be_col[:], be_d[:])
    nc.gpsimd.memset(nshift[:], -SHIFT)
    for t8 in range(4, NIC):
        nc.gpsimd.dma_start(xfb[1][:, bass.ts(t8, IC)],
                            x_d[128:256, bass.ts(t8, IC)])
    gam = cpool.tile([128, 1], F32, name="gam")
    nc.gpsimd.dma_start(gam[:], gam_d[:].to_broadcast((128, 1)))
    gbv = cpool.tile([128, 2], F32, name="gbv")
    nc.gpsimd.dma_start(gbv[:].rearrange("p (cc m) -> p cc m", cc=2),
                        gbv_d.rearrange("(cc p) m -> p cc m", cc=2))

    # ---- weight casts (DVE) ----
    wq4 = cpool.tile([128, 2 * 128], BF16, name="wq4")
    nc.vector.tensor_copy(wq4[:], wq4f[:])
    wk4 = cpool.tile([128, 2 * 128], BF16, name="wk4")
    nc.vector.tensor_copy(wk4[:], wk4f[:])
    wv = cpool.tile([128, 2 * C], BF16, name="wv")
    nc.vector.tensor_copy(wv[:], wvf[:])

    # ---- label branch (no PE): softmax(label) @ We.T + be, GpSimd broadcast ----
    if "nolabel" in KV:
        # consume the loaded tiles, write zeros to the u rows
        zrow = work.tile([LC, IC], BF16, name="zrow")
        nc.vector.memset(zrow[:], 0.0)
        nc.vector.tensor_scalar_add(zrow[0:LC, 0:LC], lbl_bc[:], 0.0)
        nc.vector.tensor_scalar_add(zrow[0:LC, LC:2 * LC], we32[:], 0.0)
        nc.vector.tensor_scalar_add(zrow[0:LC, 2 * LC:2 * LC + 1], be_col[:], 0.0)
        for t in range(NIC):
            nc.sync.dma_start(out_d[C:COUT, bass.ts(t, IC)], zrow[:])
    else:
        lmax = work.tile([LC, 1], F32, name="lmax")
        nc.vector.reduce_max(lmax[:], lbl_bc[:], axis=mybir.AxisListType.X)
        nlmax = work.tile([LC, 1], F32, name="nlmax")
        nc.vector.tensor_scalar_mul(nlmax[:], lmax[:], -1.0)
        lexp = work.tile([LC, LC], F32, name="lexp")
        nc.scalar.activation(lexp[:], lbl_bc[:], AF.Exp, bias=nlmax[:], scale=1.0)
        lsum = work.tile([LC, 1], F32, name="lsum")
        nc.vector.reduce_sum(lsum[:], lexp[:], axis=mybir.AxisListType.X)
        lrec = work.tile([LC, 1], F32, name="lrec")
        nc.vector.reciprocal(lrec[:], lsum[:])
        sjunk = work.tile([LC, LC], F32, name="sjunk")
        u_sb = work.tile([LC, 1], F32, name="u_sb")
        # u[o] = be[o] + (1/lsum) * sum_c lexp[c] * We[o, c]
        u_raw = work.tile([LC, 1], F32, name="u_raw")
        nc.vector.tensor_tensor(sjunk[:], lexp[:], we32[:], op=ALU.mult)
        nc.vector.reduce_sum(u_raw[:], sjunk[:], axis=mybir.AxisListType.X)
        nc.vector.scalar_tensor_tensor(u_sb[:], u_raw[:], lrec[:], be_col[:],
                                       op0=ALU.mult, op1=ALU.add)
        u_bc = work.tile([LC, IC], BF16, name="u_bc")
        nc.gpsimd.tensor_scalar(u_bc[:], warm_src[0:LC, 0:IC], 0.0, u_sb[:],
                                op0=ALU.mult, op1=ALU.add)
        for t in range(NIC):
            nc.sync.dma_start(out_d[C:COUT, bass.ts(t, IC)], u_bc[:])



    # ---- vt pair tiles (bf16 [128, 2*C]: vt[u][p, t*C+c] = v[c, (2u+t)*128+p]) ----
    vt = [work.tile([128, 2 * C], BF16, name=f"vt{u}") for u in range(NPAIR)]
    q4 = work.tile([128, N], BF16, name="q4")

    pt_cur = []   # 16 tiles per chunk: pair u holds j-blocks (2u, 2u+1)

    def emit_energy_pair(ic, u, pt_list):
        """Energy for j-blocks (2u, 2u+1) x i-chunk ic; exp -> fp8e5 pt tile.

        K=32 row-tiled matmuls: j-block jb runs in 32-row partition group
        jb%4, so consecutive energy matmuls occupy distinct row groups and
        execute concurrently in the PE array."""
        e_ps = pspool.tile([128, 1024], F32, name="e_ps", tag="eps", bufs=2)
        for gh in range(2):
            jb = 2 * u + gh
            g = jb % 4
            nc.tensor.matmul(e_ps[:, bass.ts(gh, IC)],
                             k_pad[32 * g:32 * (g + 1), bass.ts(jb, 128)],
                             q4[32 * g:32 * (g + 1), bass.ts(ic, IC)],
                             start=True, stop=True,
                             tile_position=(32 * g, 0))
        pt = work.tile([128, 1024], BF16, name="pt", tag="pt", bufs=32)
        nc.scalar.activation(pt[:], e_ps[:], AF.Exp, bias=nshift[:], scale=1.0)
        pt_list.append(pt)

    # ---- prologue: per-chunk casts, q/k proj, v proj, energy for ic=0 ----
    for t in range(NIC):
        tsl = bass.ts(t, IC)
        q_ps = pspool.tile([128, IC], F32, name="q_ps", tag="ops", bufs=3)
        k_ps = pspool.tile([128, IC], F32, name="k_ps", tag="ops", bufs=3)
        for kc in range(2):
            nc.tensor.matmul(q_ps[:], wq4[:, bass.ts(kc, 128)],
                             xfb[kc][:, tsl], start=(kc == 0), stop=(kc == 1))
        for kc in range(2):
            nc.tensor.matmul(k_ps[:], wk4[:, bass.ts(kc, 128)],
                             xfb[kc][:, tsl], start=(kc == 0), stop=(kc == 1))
        nc.vector.tensor_scalar_add(q4[:, tsl], q_ps[:], bq4[:])
        nc.vector.tensor_scalar_add(k_pad[:, tsl], k_ps[:], bk4[:])
        for jb in range(4 * t, 4 * t + 4):
            v_ps = pspool.tile([128, C], F32, name="v_ps", tag="lps", bufs=1)
            for kc in range(2):
                nc.tensor.matmul(v_ps[:], xfb[kc][:, bass.ts(jb, 128)],
                                 wv[:, bass.ts(kc, C)],
                                 start=(kc == 0), stop=(kc == 1))
            nc.vector.tensor_copy(vt[jb // 2][:, bass.ts(jb % 2, C)], v_ps[:])
        for u in (2 * t, 2 * t + 1):
            emit_energy_pair(0, u, pt_cur)

    # ---- steady loop: AV(ic) + l(ic) interleaved with energy/exp(ic+1) ----
    for ic in range(NIC):
        isl = bass.ts(ic, IC)
        pt_next = []
        o_ps = [pspool.tile([128, IC], F32, name=f"o_ps{h}", tag="ops", bufs=3)
                for h in range(2)]
        l_ps = pspool.tile([128, IC], F32, name="l_ps", tag="lps", bufs=1)
        for u in range(NPAIR):
            pt_u = pt_cur[u]
            for t in range(2):
                st = (u == 0 and t == 0)
                sp = (u == NPAIR - 1 and t == 1)
                ptm = pt_u[:, bass.ts(t, IC)]
                for h in range(2):
                    nc.tensor.matmul(o_ps[h][:],
                                     vt[u][:, t * C + h * 128:
                                           t * C + (h + 1) * 128],
                                     ptm, start=st, stop=sp)
                nc.tensor.matmul(l_ps[:], warm_src[:, 0:128], ptm,
                                 start=st, stop=sp)
            if ic + 1 < NIC and u % 2 == 1:
                # two pairs back-to-back: 4 row-tiled K=32 matmuls covering
                # row groups 0-3 execute concurrently in the PE array
                emit_energy_pair(ic + 1, u - 1, pt_next)
                emit_energy_pair(ic + 1, u, pt_next)
        # postproc: out[c,i] = gamma/l * o_ps + (gamma*bv_c + xf)
        # (clamps kept as cheap guards against any overflow columns)
        o_cl = []
        for h in range(2):
            oc = work.tile([128, IC], F32, name="o_cl", tag="ocl", bufs=4)
            nc.vector.tensor_scalar(oc[:], o_ps[h][:], 1e18, -1e18,
                                    op0=ALU.min, op1=ALU.max)
            o_cl.append(oc)
        l_sb = work.tile([128, IC], F32, name="l_sb", tag="lsb", bufs=2)
        nc.vector.tensor_scalar(l_sb[:], l_ps[:], 1e30, EPS,
                                op0=ALU.min, op1=ALU.add)
        rec = work.tile([128, IC], F32, name="rec", tag="rec", bufs=2)
        nc.vector.reciprocal_approx_fast(out=rec[:], in_=l_sb[:])
        for h in range(2):
            tmp = work.tile([128, IC], F32, name="tmp", tag="tmp", bufs=2)
            nc.vector.scalar_tensor_tensor(tmp[:], o_cl[h][:], gam[:],
                                           rec[:],
                                           op0=ALU.mult, op1=ALU.mult)
            obs = work.tile([128, IC], BF16, name="obs", tag="obs", bufs=4)
            nc.vector.scalar_tensor_tensor(obs[:], tmp[:], gbv[:, h:h + 1],
                                           xfb[h][:, isl],
                                           op0=ALU.add, op1=ALU.add)
            eng = nc.sync if h == 0 else nc.gpsimd
            eng.dma_start(out_d[h * 128:(h + 1) * 128, isl], obs[:])
        pt_cur = pt_next

    ctx.close()


_COMPILED = None
_COMPILED_FAST = None


def _get_compiled():
    global _COMPILED
    if _COMPILED is None:
        nc = bacc.Bacc("TRN2", target_bir_lowering=False, debug=False)
        with tile.TileContext(nc) as tc:
            build_program(nc, tc)
        nc.compile()
        _COMPILED = nc
    return _COMPILED


def _get_compiled_fast():
    global _COMPILED_FAST
    if _COMPILED_FAST is None:
        nc = bacc.Bacc("TRN2", target_bir_lowering=False, debug=False)
        with tile.TileContext(nc) as tc:
            build_fast_program(nc, tc)
        nc.compile()
        _COMPILED_FAST = nc
    return _COMPILED_FAST


def _fast_output_ok(res, in_maps, label, We, be):
    """Sanity-check the gamma==0 device output (guards device transients)."""
    try:
        for b in range(B):
            o = np.asarray(res.results[b]["out"])
            if not np.array_equal(o[0:C], in_maps[b]["x"]):
                return False
            s = np.exp(label[b] - label[b].max())
            u = We.astype(np.float64) @ (s / s.sum()) + be
            du = np.abs(o[C:COUT].astype(np.float32) - u[:, None].astype(np.float32))
            if not np.all(du <= 0.05 + 0.1 * np.abs(u)[:, None]):
                return False
        return True
    except Exception:
        return False


def kernel(x, label, Wq, bq, Wk, bk, Wv, bv, gamma, We, be, _trace=False):
    x = np.asarray(x, np.float32)
    label = np.asarray(label, np.float32)
    Wq, bq = np.asarray(Wq, np.float32), np.asarray(bq, np.float32)
    Wk, bk = np.asarray(Wk, np.float32), np.asarray(bk, np.float32)
    Wv, bv = np.asarray(Wv, np.float32), np.asarray(bv, np.float32)
    gamma = np.asarray(gamma, np.float32)
    We, be = np.asarray(We, np.float32), np.asarray(be, np.float32)

    if np.all(gamma == 0.0):
        # out = gamma*attn + x degenerates to x: skip the attention pipeline
        nc = _get_compiled_fast()
        in_maps = [host_prep_fast(x[b], label[b], We, be) for b in range(B)]
        # the fast path is exactly checkable on host (rows 0:C are a bit-exact
        # DMA copy; u rows are 32 scalars) -- retry once on a transient flake
        for _attempt in range(2):
            res = run_bass_kernel_spmd(nc, in_maps, list(range(B)),
                                       trace=_trace)
            if _fast_output_ok(res, in_maps, label, We, be):
                break
    else:
        nc = _get_compiled()
        in_maps = [host_prep(x[b], label[b], Wq, bq, Wk, bk, Wv, bv, gamma,
                             We, be)
                   for b in range(B)]
        res = run_bass_kernel_spmd(nc, in_maps, list(range(B)), trace=_trace)
    out = np.stack([res.results[b]["out"] for b in range(B)])
    out = out.reshape(B, COUT, HW, HW).astype(np.float32)
    if _trace:
        return out, res
    return out

